# revision 3
# baseline (speedup 1.0000x reference)
"""Trainium2 Bass kernel for CustomConvWithExtra.

Problem: out = conv3x3(x, w_main) + b_main + extra, where extra is a conv of a
spatially-constant per-(b,c) image -> collapses to a 3x3 border-class table
T[b,c,clsh,clsw] (interior/edge/corner values).

Strategy:
 - Data parallel: 1 batch image per NeuronCore (B=8 = 8 cores), weights replicated.
 - Conv as matmul: per output row PAIR, one matmul with stationary
   lhsT [57,128] (block-diag: 2 row-blocks x 27 taps -> 2x64 channels) and
   moving rhs [57, 512] im2col patch, accumulated in PSUM [128,512].
 - Rows 54/55/56 of the patch are static (w==0 indicator, w==511 indicator,
   ones); the matching lhsT rows carry the border-delta and base-bias terms so
   the whole "extra + bias" is fused into the same matmul.
 - float32r operands: full-rate PE (1 cycle/row) with reduced-precision
   multiply, fp32 accumulate.
 - Patches are built by strided DMA from a host-padded xp [3,514,514] in DRAM.
 - PSUM -> SBUF copy alternates Vector/Scalar engines; output leaves in
   multi-megabyte DMAs (C row-pairs per DMA).
"""

from contextlib import ExitStack

import numpy as np

import concourse.bass as bass
import concourse.tile as tile
from concourse import bacc, mybir
from concourse.bass_utils import run_bass_kernel_spmd

# Problem shapes (hardcoded per contract)
B, CIN, H, W = 8, 3, 512, 512
COUT, E, KS = 64, 3, 3
NCORES = 8
KP = 57            # patch partitions: 54 = 2 pairs x 27 taps, + indL + indR + ones
C = 16             # row-pairs per chunk
F32R = mybir.dt.float32r
F32 = mybir.dt.float32

_cache: dict = {}


def _build(h: int = H, w: int = W):
    """Build + compile the per-core Bass program (SPMD, identical on all cores)."""
    pairs = h // 2
    c = min(C, pairs)
    nchunk = pairs // c
    assert pairs % c == 0
    xrow = w + 2

    nc = bacc.Bacc("TRN2", target_bir_lowering=False, debug=False)
    xp = nc.dram_tensor("xp", [CIN, h + 2, w + 2], F32R, kind="ExternalInput").ap()
    wts = nc.dram_tensor("wts", [3, KP, 128], F32R, kind="ExternalInput").ap()
    stat = nc.dram_tensor("stat", [3, c * w], F32R, kind="ExternalInput").ap()
    out = nc.dram_tensor("out", [COUT, h, w], F32, kind="ExternalOutput").ap()

    PBUFS = 3
    with tile.TileContext(nc) as tc, ExitStack() as ctx:
        wpool = ctx.enter_context(tc.tile_pool(name="wpool", bufs=1))
        ppool = ctx.enter_context(tc.tile_pool(name="ppool", bufs=PBUFS))
        opool = ctx.enter_context(tc.tile_pool(name="opool", bufs=2))
        pspool = ctx.enter_context(tc.tile_pool(name="pspool", bufs=4, space="PSUM"))

        # Stationary weights: wtile[k, v*128+m] = wts[v, k, m]
        wtile = wpool.tile([KP, 3 * 128], F32R)
        nc.sync.dma_start(
            wtile[:, :], bass.AP(wts.tensor, 0, [[128, KP], [KP * 128, 3], [1, 128]])
        )

        # Patch buffers; static rows 54:57 loaded once per physical buffer.
        patch_tiles = []
        for s in range(PBUFS):
            pt = ppool.tile([KP, c * w], F32R, name=f"patch{s}", tag="patch")
            nc.sync.dma_start(pt[54:57, :], stat[:, :])
            patch_tiles.append(pt)

        for ch in range(nchunk):
            pt = patch_tiles[ch % PBUFS]
            h0 = ch * c * 2
            # Fill data rows 0..53: one DMA per (pair, ci, kh) covering 3 kw rows.
            for pair in range(2):
                for ci in range(CIN):
                    for kh in range(3):
                        p0 = pair * 27 + ci * 9 + kh * 3
                        src = bass.AP(
                            xp.tensor,
                            ci * xrow * (h + 2) + (h0 + pair + kh) * xrow,
                            [[1, 3], [2 * xrow, c], [1, w]],
                        )
                        nc.sync.dma_start(pt[p0 : p0 + 3, :], src)

            ob = opool.tile([128, c * w], F32, name="ob", tag="ob")
            for j in range(c):
                ps = pspool.tile([128, w], F32, name="ps", tag="ps")
                pairidx = ch * c + j
                v = 0 if pairidx == 0 else (2 if pairidx == pairs - 1 else 1)
                nc.tensor.matmul(
                    ps[:, :],
                    wtile[:, v * 128 : (v + 1) * 128],
                    pt[:, j * w : (j + 1) * w],
                    start=True,
                    stop=True,
                )
                if j % 2 == 0:
                    nc.vector.tensor_copy(ob[:, j * w : (j + 1) * w], ps[:, :])
                else:
                    nc.scalar.copy(ob[:, j * w : (j + 1) * w], ps[:, :])

            for pair in range(2):
                dst = bass.AP(
                    out.tensor,
                    (h0 + pair) * w,
                    [[h * w, COUT], [2 * w, c], [1, w]],
                )
                nc.sync.dma_start(dst, ob[pair * 64 : (pair + 1) * 64, :])

    nc.compile()
    return nc


def _host_prep(x, v, wm, bm, we, be, h=H, w=W, c=C):
    """Per-core inputs: padded image, fused weight variants, static patch rows."""
    Bb = x.shape[0]
    vr = v.reshape(Bb, COUT, E).astype(np.float64)

    # Border-class tap sums of w_extra: cls 0=first(top/left) 1=mid 2=last
    sets = {0: [1, 2], 1: [0, 1, 2], 2: [0, 1]}
    Mcl = np.zeros((COUT, E, 3, 3), np.float64)
    we64 = we.astype(np.float64)
    for ch_ in range(3):
        for cw in range(3):
            Mcl[:, :, ch_, cw] = we64[:, :, sets[ch_], :][:, :, :, sets[cw]].sum((2, 3))
    # T[b,c,clsh,clsw] = extra-term value + both biases
    T = (
        np.einsum("bce,cehw->bchw", vr, Mcl)
        + bm.astype(np.float64)[None, :, None, None]
        + be.astype(np.float64)[None, :, None, None]
    )

    xp = np.pad(x, ((0, 0), (0, 0), (1, 1), (1, 1))).astype(np.float32)

    wts = np.zeros((Bb, 3, KP, 128), np.float32)
    for b in range(Bb):
        for vi, clss in enumerate([(0, 1), (1, 1), (1, 2)]):
            for pair in range(2):
                cls = clss[pair]
                cols = slice(pair * 64, pair * 64 + 64)
                for ci in range(CIN):
                    for kh in range(KS):
                        for kw in range(KS):
                            wts[b, vi, pair * 27 + ci * 9 + kh * 3 + kw, cols] = wm[
                                :, ci, kh, kw
                            ]
                wts[b, vi, 54, cols] = T[b, :, cls, 0] - T[b, :, cls, 1]
                wts[b, vi, 55, cols] = T[b, :, cls, 2] - T[b, :, cls, 1]
                wts[b, vi, 56, cols] = T[b, :, cls, 1]

    stat = np.zeros((3, c * w), np.float32)
    stat[0, 0 :: w] = 1.0   # w == 0 indicator
    stat[1, w - 1 :: w] = 1.0  # w == W-1 indicator
    stat[2, :] = 1.0        # ones row (base bias)
    return xp, wts, stat


def kernel(**inputs) -> np.ndarray:
    x = np.ascontiguousarray(np.asarray(inputs["x"], np.float32))
    v = np.asarray(inputs["extra_inputs"], np.float32)
    wm = np.asarray(inputs["w_main"], np.float32)
    bm = np.asarray(inputs["b_main"], np.float32)
    we = np.asarray(inputs["w_extra"], np.float32)
    be = np.asarray(inputs["b_extra"], np.float32)

    xp, wts, stat = _host_prep(x, v, wm, bm, we, be)

    if "nc" not in _cache:
        _cache["nc"] = _build()
    nc = _cache["nc"]

    in_maps = [
        {"xp": xp[b], "wts": wts[b], "stat": stat} for b in range(B)
    ]
    res = run_bass_kernel_spmd(nc, in_maps, list(range(NCORES)))
    return np.stack([res.results[b]["out"] for b in range(B)]).astype(np.float32)
